# revision 8
# baseline (speedup 1.0000x reference)
"""ClusterInversionLoss Trainium2 kernel (v3).

Strategy (data-parallel over the flat pair list, per sharding hint):
  - Host: gather rows at pair_i/pair_j, drop inactive pairs exactly
    (y_i == y_j contributes 0 to both loss and weight), fold the sign
    by swapping pair sides so sign == +1 always, precompute
    DW = dist*(wi+wj) and WA = (wi+wj) per pair.  Shard the active
    pairs across 8 cores; chunk-major DRAM layout keeps every
    partition's DMA contiguous.
  - Device per chunk.  The GpSimd engine is left idle on purpose: it
    shares an exclusive SBUF port with DVE, so concurrent Pool work
    roughly halves combined elementwise throughput (measured).  Work
    splits between ACT and DVE only, with PE taking the reductions:
      ACT : one exp over all 8 logit planes; side-i reciprocal as
            ln(1+T) via the free affine bias then exp(-x) (no +1 add,
            all within the one pinned exp+ln table set); softplus
            exp(-d) / ln(1+x) batched over chunk pairs.
      DVE : Z/W suffix-sum chains for both sides via 6 double-side
            [P,2,LC] bf16 tensor_tensor adds (2x mode); side-j
            reciprocal via (T+1) in f32 + reciprocal_approx_fast
            (single DVE op, ~18 bits); the s multiplies, d subtract,
            and the SP*DW multiply (2x).
      PE  : ones-matmul partition reductions of the per-pair loss and
            weight planes, accumulated across chunks in two PSUM
            banks; evicted once at the end (PSUM -> DVE -> SBUF).
  - Host: sum the 8 x [1, 2*LC] partials, return loss/weight ratio.

Computes exactly the reference quantity: inactive pairs contribute 0,
the 0.5 pair-weight factor cancels in the ratio, zero-padding lands
on DW = WA = 0.
"""

import numpy as np

import concourse.bacc as bacc
import concourse.mybir as mybir
from concourse.bass_utils import run_bass_kernel_spmd
from concourse.tile import TileContext

NCORES = 8
P = 128
NCHUNK = 4
LC = 400
L = NCHUNK * LC            # 1600 columns per partition
PC = P * L                 # 204,800 pair slots per core
CAP = NCORES * PC          # 1,638,400 total slots (>= ~1.585M active)

EPS = 1e-8

f32 = mybir.dt.float32
bf16 = mybir.dt.bfloat16
f8 = mybir.dt.float8e4
AF = mybir.ActivationFunctionType
ALU = mybir.AluOpType


def _pin_act_tables(arch):
    """Make every ACT function we use first-match to the one table set
    containing both exp and ln, so a single ACT_TABLE_LOAD suffices."""
    from concourse.hw_specs import get_activation_tables

    tabs = get_activation_tables(arch)
    ours = {AF.Exp, AF.Ln}
    combined = None
    for name, fns in tabs.items():
        if ours <= fns:
            combined = name
            break
    if combined is None:
        return
    for name, fns in tabs.items():
        if name != combined:
            fns -= ours


def _build():
    nc = bacc.Bacc("TRN2", target_bir_lowering=False)
    _pin_act_tables(nc.m.arch)
    XL = nc.dram_tensor("xl", [NCHUNK, P, 8, LC], f8, kind="ExternalInput")
    XD = nc.dram_tensor("xd", [NCHUNK, P, LC], bf16, kind="ExternalInput")
    XA = nc.dram_tensor("xa", [NCHUNK, P, LC], bf16, kind="ExternalInput")
    OUT = nc.dram_tensor("out", [1, 2 * LC], f32, kind="ExternalOutput")

    with TileContext(nc) as tc:
        with (
            tc.tile_pool(name="io", bufs=NCHUNK) as io,
            tc.tile_pool(name="ew", bufs=3) as ew,
            tc.tile_pool(name="sc", bufs=3) as sc,
            tc.tile_pool(name="s1", bufs=2) as s1,
            tc.tile_pool(name="c0", bufs=1) as c0,
            tc.psum_pool(name="ps", bufs=1) as psp,
        ):
            ONE1 = c0.tile([P, 1], bf16, tag="ONE1", name="ONE1")
            nc.vector.memset(ONE1[:], 1.0)
            DW4 = c0.tile([P, NCHUNK, LC], bf16, tag="DW4", name="DW4")
            psL = psp.tile([1, LC], f32, tag="psL", name="psL")
            psW = psp.tile([1, LC], f32, tag="psW", name="psW")
            PAIR = [{} for _ in range(NCHUNK // 2)]

            LGs = []
            WAs = []
            for c in range(NCHUNK):
                LG = io.tile([P, 8, LC], f8, tag="LG", name="LG")
                nc.sync.dma_start(out=LG[:], in_=XL[c])
                WA = io.tile([P, LC], bf16, tag="WA", name="WA")
                nc.sync.dma_start(out=WA[:], in_=XA[c])
                nc.sync.dma_start(out=DW4[:, c, :], in_=XD[c])
                LGs.append(LG)
                WAs.append(WA)

            def front(c):
                """exp, Z/W sums, both reciprocals, s, d, WA mm."""
                nc.tensor.matmul(psW[:], ONE1[:], WAs[c][:],
                                 start=(c == 0), stop=(c == NCHUNK - 1))

                E = ew.tile([P, 8, LC], bf16, tag="E", name="E")
                nc.scalar.activation(E[:], LGs[c][:], AF.Exp)

                # planes class-major, side-interleaved:
                # [i1, j1, i2, j2, i3, j3, i4, j4]
                e1, e2, e3, e4 = (E[:, 0:2, :], E[:, 2:4, :],
                                  E[:, 4:6, :], E[:, 6:8, :])
                A = sc.tile([P, 2, LC], bf16, tag="A", name="A")
                B = sc.tile([P, 2, LC], bf16, tag="B", name="B")
                T = sc.tile([P, 2, LC], bf16, tag="T", name="T")
                U = sc.tile([P, 2, LC], bf16, tag="U", name="U")
                V = sc.tile([P, 2, LC], bf16, tag="V", name="V")
                W = sc.tile([P, 2, LC], bf16, tag="W", name="W")
                nc.vector.tensor_add(out=A[:], in0=e3, in1=e4)
                nc.vector.tensor_add(out=B[:], in0=e2, in1=A[:])
                nc.vector.tensor_add(out=T[:], in0=e1, in1=B[:])
                nc.vector.tensor_add(out=U[:], in0=T[:], in1=B[:])
                nc.vector.tensor_add(out=V[:], in0=A[:], in1=e4)
                nc.vector.tensor_add(out=W[:], in0=U[:], in1=V[:])

                # side i: reciprocal on ACT -- ln(1+T) via bias, exp(-x)
                LZ = sc.tile([P, LC], f32, tag="LZ", name="LZ")
                nc.scalar.activation(LZ[:], T[:, 0, :], AF.Ln, bias=1.0)
                RZi = sc.tile([P, LC], bf16, tag="RZi", name="RZi")
                nc.scalar.activation(RZi[:], LZ[:], AF.Exp, scale=-1.0)
                # side j: reciprocal on DVE -- (T+1) f32, approx recip
                Zj = sc.tile([P, LC], f32, tag="Zj", name="Zj")
                nc.vector.tensor_scalar_add(out=Zj[:], in0=T[:, 1, :],
                                            scalar1=1.0)
                RZj = sc.tile([P, LC], f32, tag="RZj", name="RZj")
                nc.vector.reciprocal_approx_fast(out=RZj[:], in_=Zj[:])

                Si = sc.tile([P, LC], bf16, tag="Si", name="Si")
                nc.vector.tensor_mul(out=Si[:], in0=W[:, 0, :], in1=RZi[:])
                Sj = sc.tile([P, LC], bf16, tag="Sj", name="Sj")
                nc.vector.tensor_mul(out=Sj[:], in0=W[:, 1, :], in1=RZj[:])

                pr = PAIR[c // 2]
                if c % 2 == 0:
                    DD2 = s1.tile([P, 2, LC], bf16, tag="DD2", name="DD2")
                    pr["DD2"] = DD2
                nc.vector.tensor_sub(out=pr["DD2"][:, c % 2, :],
                                     in0=Si[:], in1=Sj[:])

            def back(k):
                """softplus over a chunk pair, SP*DW, PE loss reduce."""
                DD2 = PAIR[k]["DD2"]
                G = s1.tile([P, 2, LC], bf16, tag="G", name="G")
                nc.scalar.activation(G[:], DD2[:], AF.Exp, scale=-1.0)
                SP = s1.tile([P, 2, LC], bf16, tag="SP", name="SP")
                nc.scalar.activation(SP[:], G[:], AF.Ln, bias=1.0)
                LP = s1.tile([P, 2, LC], bf16, tag="LP", name="LP")
                nc.vector.tensor_mul(out=LP[:], in0=SP[:],
                                     in1=DW4[:, 2 * k:2 * k + 2, :])
                for h in (0, 1):
                    c = 2 * k + h
                    nc.tensor.matmul(psL[:], ONE1[:], LP[:, h, :],
                                     start=(c == 0), stop=(c == NCHUNK - 1))

            for c in range(NCHUNK):
                front(c)
                if c % 2 == 1:
                    back(c // 2)

            RES = c0.tile([1, 2 * LC], f32, tag="RES", name="RES")
            nc.vector.tensor_copy(out=RES[:, 0:LC], in_=psL[:])
            nc.vector.tensor_copy(out=RES[:, LC:2 * LC], in_=psW[:])
            nc.sync.dma_start(out=OUT[:], in_=RES[:])

    nc.compile()
    return nc


_NC_CACHE = {}


def _get_nc():
    if "nc" not in _NC_CACHE:
        _NC_CACHE["nc"] = _build()
    return _NC_CACHE["nc"]


def _prepare(inputs, targets, cluster_ids, sample_weight, pair_i, pair_j):
    import ml_dtypes

    x = np.ascontiguousarray(np.asarray(inputs), dtype=np.float32)
    t = np.asarray(targets)
    w = np.asarray(sample_weight, dtype=np.float32)
    pi = np.asarray(pair_i).astype(np.int64, copy=False)
    pj = np.asarray(pair_j).astype(np.int64, copy=False)

    yi = t[pi]
    yj = t[pj]
    dy = (yi - yj).astype(np.int64)
    act = dy != 0
    # fold the sign: swap sides where y_i < y_j, so delta = s_i - s_j
    swap = dy < 0
    pi2 = np.where(swap, pj, pi)[act]
    pj2 = np.where(swap, pi, pj)[act]
    dist = np.abs(dy[act]).astype(np.float32)
    n = pi2.shape[0]
    assert n <= CAP, f"active pairs {n} exceed capacity {CAP}"

    li = x[pi2]                       # (n, 5)
    lj = x[pj2]
    lis = li[:, 1:5] - li[:, 0:1]     # l0-shift: softmax shift-invariant
    ljs = lj[:, 1:5] - lj[:, 0:1]
    ws = w[pi2] + w[pj2]              # 2*w_pair; the 2 cancels in the ratio

    f8np = ml_dtypes.float8_e4m3fn
    bf = ml_dtypes.bfloat16
    L8 = np.zeros((CAP, 8), dtype=f8np)
    L8[:n, 0::2] = lis.astype(f8np)
    L8[:n, 1::2] = ljs.astype(f8np)
    WD = np.zeros((CAP,), dtype=bf)
    WD[:n] = (dist * ws).astype(bf)
    WS = np.zeros((CAP,), dtype=bf)
    WS[:n] = ws.astype(bf)

    # slot -> (core, chunk, partition, col); plane axis moved before col
    XLs = np.ascontiguousarray(
        L8.reshape(NCORES, NCHUNK, P, LC, 8).transpose(0, 1, 2, 4, 3))
    XDs = WD.reshape(NCORES, NCHUNK, P, LC)
    XAs = WS.reshape(NCORES, NCHUNK, P, LC)
    return [{"xl": XLs[k], "xd": XDs[k], "xa": XAs[k]} for k in range(NCORES)]


def _run(in_maps, trace=False, **kw):
    nc = _get_nc()
    return run_bass_kernel_spmd(nc, in_maps, list(range(NCORES)), trace=trace, **kw)


def kernel(inputs, targets, cluster_ids, sample_weight, pair_i, pair_j):
    in_maps = _prepare(inputs, targets, cluster_ids, sample_weight, pair_i, pair_j)
    res = _run(in_maps)
    tl = 0.0
    tw = 0.0
    for k in range(NCORES):
        o = res.results[k]["out"]
        tl += float(o[0, 0:LC].sum(dtype=np.float64))
        tw += float(o[0, LC:2 * LC].sum(dtype=np.float64))
    # the 0.5 pair-weight factor cancels in the ratio; fold it into eps
    return np.float32(tl / (tw + 2 * EPS))


# revision 9
# speedup vs baseline: 1.1603x; 1.1603x over previous
"""ClusterInversionLoss Trainium2 kernel (v3).

Strategy (data-parallel over the flat pair list, per sharding hint):
  - Host: gather rows at pair_i/pair_j, drop inactive pairs exactly
    (y_i == y_j contributes 0 to both loss and weight), fold the sign
    by swapping pair sides so sign == +1 always, precompute
    DW = dist*(wi+wj) and WA = (wi+wj) per pair.  Shard the active
    pairs across 8 cores; chunk-major DRAM layout keeps every
    partition's DMA contiguous.
  - Device per chunk.  The GpSimd engine is left idle on purpose: it
    shares an exclusive SBUF port with DVE, so concurrent Pool work
    roughly halves combined elementwise throughput (measured).  Work
    splits between ACT and DVE only, with PE taking the reductions:
      ACT : one exp over all 8 logit planes; side-i reciprocal as
            ln(1+T) via the free affine bias then exp(-x) (no +1 add,
            all within the one pinned exp+ln table set); softplus
            exp(-d) / ln(1+x) batched over chunk pairs.
      DVE : Z/W suffix-sum chains for both sides via 6 double-side
            [P,2,LC] bf16 tensor_tensor adds (2x mode); side-j
            reciprocal via (T+1) in f32 + reciprocal_approx_fast
            (single DVE op, ~18 bits); the s multiplies, d subtract,
            and the SP*DW multiply (2x).
      PE  : ones-matmul partition reductions of the per-pair loss and
            weight planes, accumulated across chunks in two PSUM
            banks; evicted once at the end (PSUM -> DVE -> SBUF).
  - Host: sum the 8 x [1, 2*LC] partials, return loss/weight ratio.

Computes exactly the reference quantity: inactive pairs contribute 0,
the 0.5 pair-weight factor cancels in the ratio, zero-padding lands
on DW = WA = 0.
"""

import numpy as np

import concourse.bacc as bacc
import concourse.mybir as mybir
from concourse.bass_utils import run_bass_kernel_spmd
from concourse.tile import TileContext

NCORES = 8
P = 128
NCHUNK = 4
LC = 400
L = NCHUNK * LC            # 1600 columns per partition
PC = P * L                 # 204,800 pair slots per core
CAP = NCORES * PC          # 1,638,400 total slots (>= ~1.585M active)

EPS = 1e-8

f32 = mybir.dt.float32
bf16 = mybir.dt.bfloat16
f8 = mybir.dt.float8e4
AF = mybir.ActivationFunctionType
ALU = mybir.AluOpType


def _pin_act_tables(arch):
    """Make every ACT function we use first-match to the one table set
    containing both exp and ln, so a single ACT_TABLE_LOAD suffices."""
    from concourse.hw_specs import get_activation_tables

    tabs = get_activation_tables(arch)
    ours = {AF.Exp, AF.Ln}
    combined = None
    for name, fns in tabs.items():
        if ours <= fns:
            combined = name
            break
    if combined is None:
        return
    for name, fns in tabs.items():
        if name != combined:
            fns -= ours


def _build():
    nc = bacc.Bacc("TRN2", target_bir_lowering=False)
    _pin_act_tables(nc.m.arch)
    XL = nc.dram_tensor("xl", [NCHUNK, P, 8, LC], f8, kind="ExternalInput")
    XD = nc.dram_tensor("xd", [NCHUNK, P, LC], bf16, kind="ExternalInput")
    XA = nc.dram_tensor("xa", [NCHUNK, P, LC], bf16, kind="ExternalInput")
    OUT = nc.dram_tensor("out", [1, 2 * LC], f32, kind="ExternalOutput")

    with TileContext(nc) as tc:
        with (
            tc.tile_pool(name="io", bufs=NCHUNK) as io,
            tc.tile_pool(name="ew", bufs=3) as ew,
            tc.tile_pool(name="sc", bufs=3) as sc,
            tc.tile_pool(name="s1", bufs=2) as s1,
            tc.tile_pool(name="c0", bufs=1) as c0,
            tc.psum_pool(name="ps", bufs=1) as psp,
        ):
            ONE1 = c0.tile([P, 1], bf16, tag="ONE1", name="ONE1")
            nc.vector.memset(ONE1[:], 1.0)
            DW4 = c0.tile([P, NCHUNK, LC], bf16, tag="DW4", name="DW4")
            psL = psp.tile([1, LC], f32, tag="psL", name="psL")
            psW = psp.tile([1, LC], f32, tag="psW", name="psW")
            PAIR = [{} for _ in range(NCHUNK // 2)]

            LGs = []
            WAs = []
            for c in range(NCHUNK):
                LG = io.tile([P, 8, LC], f8, tag="LG", name="LG")
                nc.sync.dma_start(out=LG[:], in_=XL[c])
                WA = io.tile([P, LC], bf16, tag="WA", name="WA")
                nc.sync.dma_start(out=WA[:], in_=XA[c])
                nc.sync.dma_start(out=DW4[:, c, :], in_=XD[c])
                LGs.append(LG)
                WAs.append(WA)

            DD = {}

            def front(c):
                """exp, Z/W sums, reciprocals, s, d, WA mm."""
                nc.tensor.matmul(psW[:], ONE1[:], WAs[c][:],
                                 start=(c == 0), stop=(c == NCHUNK - 1))

                E = ew.tile([P, 8, LC], bf16, tag="E", name="E")
                nc.scalar.activation(E[:], LGs[c][:], AF.Exp)

                # planes class-major, side-interleaved:
                # [i1, j1, i2, j2, i3, j3, i4, j4]
                e1, e2, e3, e4 = (E[:, 0:2, :], E[:, 2:4, :],
                                  E[:, 4:6, :], E[:, 6:8, :])
                A = sc.tile([P, 2, LC], bf16, tag="A", name="A")
                B = sc.tile([P, 2, LC], bf16, tag="B", name="B")
                T = sc.tile([P, 2, LC], bf16, tag="T", name="T")
                U = sc.tile([P, 2, LC], bf16, tag="U", name="U")
                V = sc.tile([P, 2, LC], bf16, tag="V", name="V")
                W = sc.tile([P, 2, LC], bf16, tag="W", name="W")
                nc.vector.tensor_add(out=A[:], in0=e3, in1=e4)
                nc.vector.tensor_add(out=B[:], in0=e2, in1=A[:])
                nc.vector.tensor_add(out=T[:], in0=e1, in1=B[:])
                nc.vector.tensor_add(out=U[:], in0=T[:], in1=B[:])
                nc.vector.tensor_add(out=V[:], in0=A[:], in1=e4)
                nc.vector.tensor_add(out=W[:], in0=U[:], in1=V[:])

                if c % 2 == 0:
                    D2 = s1.tile([P, 2, LC], bf16, tag="DD2", name="DD2")
                    DD[c] = D2
                    DD[c + 1] = D2
                dst = DD[c][:, c % 2, :]

                if c < NCHUNK - 1:
                    # side i on ACT: ln(1+T) via free bias, then exp(-x)
                    LZ = sc.tile([P, LC], f32, tag="LZ", name="LZ")
                    nc.scalar.activation(LZ[:], T[:, 0, :], AF.Ln, bias=1.0)
                    RZi = sc.tile([P, LC], bf16, tag="RZi", name="RZi")
                    nc.scalar.activation(RZi[:], LZ[:], AF.Exp, scale=-1.0)
                    # side j on DVE: (T+1) f32, approx reciprocal
                    Zj = sc.tile([P, LC], f32, tag="Zj", name="Zj")
                    nc.vector.tensor_scalar_add(out=Zj[:], in0=T[:, 1, :],
                                                scalar1=1.0)
                    RZj = sc.tile([P, LC], f32, tag="RZj", name="RZj")
                    nc.vector.reciprocal_approx_fast(out=RZj[:], in_=Zj[:])
                    Si = sc.tile([P, LC], bf16, tag="Si", name="Si")
                    nc.vector.tensor_mul(out=Si[:], in0=W[:, 0, :], in1=RZi[:])
                    Sj = sc.tile([P, LC], bf16, tag="Sj", name="Sj")
                    nc.vector.tensor_mul(out=Sj[:], in0=W[:, 1, :], in1=RZj[:])
                    nc.vector.tensor_sub(out=dst, in0=Si[:], in1=Sj[:])
                else:
                    # last chunk: both sides on DVE -- no cross-engine hops
                    # on the drain path
                    Zb = sc.tile([P, 2, LC], f32, tag="Zb", name="Zb")
                    nc.vector.tensor_scalar_add(out=Zb[:], in0=T[:],
                                                scalar1=1.0)
                    RZb = sc.tile([P, 2, LC], f32, tag="RZb", name="RZb")
                    nc.vector.reciprocal_approx_fast(out=RZb[:], in_=Zb[:])
                    Sb = sc.tile([P, 2, LC], bf16, tag="Sb", name="Sb")
                    nc.vector.tensor_mul(out=Sb[:], in0=W[:], in1=RZb[:])
                    nc.vector.tensor_sub(out=dst, in0=Sb[:, 0, :],
                                         in1=Sb[:, 1, :])

            def back(c, both):
                """softplus, SP*DW, PE loss reduce; both=pair batch."""
                if both:
                    src, dwsl, cols = DD[c][:], DW4[:, c:c + 2, :], (c, c + 1)
                else:
                    src, dwsl, cols = (DD[c][:, c % 2, :],
                                       DW4[:, c, :], (c,))
                sh = [P, 2, LC] if both else [P, LC]
                G = s1.tile(sh, bf16, tag=f"G{len(cols)}", name="G")
                nc.scalar.activation(G[:], src, AF.Exp, scale=-1.0)
                SP = s1.tile(sh, bf16, tag=f"SP{len(cols)}", name="SP")
                nc.scalar.activation(SP[:], G[:], AF.Ln, bias=1.0)
                LP = s1.tile(sh, bf16, tag=f"LP{len(cols)}", name="LP")
                nc.vector.tensor_mul(out=LP[:], in0=SP[:], in1=dwsl)
                for h, cc in enumerate(cols):
                    lpsl = LP[:, h, :] if both else LP[:]
                    nc.tensor.matmul(psL[:], ONE1[:], lpsl,
                                     start=(cc == 0), stop=(cc == NCHUNK - 1))

            RES = c0.tile([1, 2 * LC], f32, tag="RES", name="RES")
            front(0)
            front(1)
            back(0, both=True)
            front(2)
            back(2, both=False)
            front(3)
            nc.vector.tensor_copy(out=RES[:, LC:2 * LC], in_=psW[:])
            back(3, both=False)
            nc.vector.tensor_copy(out=RES[:, 0:LC], in_=psL[:])
            nc.sync.dma_start(out=OUT[:], in_=RES[:])

    nc.compile()
    return nc


_NC_CACHE = {}


def _get_nc():
    if "nc" not in _NC_CACHE:
        _NC_CACHE["nc"] = _build()
    return _NC_CACHE["nc"]


def _prepare(inputs, targets, cluster_ids, sample_weight, pair_i, pair_j):
    import ml_dtypes

    x = np.ascontiguousarray(np.asarray(inputs), dtype=np.float32)
    t = np.asarray(targets)
    w = np.asarray(sample_weight, dtype=np.float32)
    pi = np.asarray(pair_i).astype(np.int64, copy=False)
    pj = np.asarray(pair_j).astype(np.int64, copy=False)

    yi = t[pi]
    yj = t[pj]
    dy = (yi - yj).astype(np.int64)
    act = dy != 0
    # fold the sign: swap sides where y_i < y_j, so delta = s_i - s_j
    swap = dy < 0
    pi2 = np.where(swap, pj, pi)[act]
    pj2 = np.where(swap, pi, pj)[act]
    dist = np.abs(dy[act]).astype(np.float32)
    n = pi2.shape[0]
    assert n <= CAP, f"active pairs {n} exceed capacity {CAP}"

    li = x[pi2]                       # (n, 5)
    lj = x[pj2]
    lis = li[:, 1:5] - li[:, 0:1]     # l0-shift: softmax shift-invariant
    ljs = lj[:, 1:5] - lj[:, 0:1]
    ws = w[pi2] + w[pj2]              # 2*w_pair; the 2 cancels in the ratio

    f8np = ml_dtypes.float8_e4m3fn
    bf = ml_dtypes.bfloat16
    L8 = np.zeros((CAP, 8), dtype=f8np)
    L8[:n, 0::2] = lis.astype(f8np)
    L8[:n, 1::2] = ljs.astype(f8np)
    WD = np.zeros((CAP,), dtype=bf)
    WD[:n] = (dist * ws).astype(bf)
    WS = np.zeros((CAP,), dtype=bf)
    WS[:n] = ws.astype(bf)

    # slot -> (core, chunk, partition, col); plane axis moved before col
    XLs = np.ascontiguousarray(
        L8.reshape(NCORES, NCHUNK, P, LC, 8).transpose(0, 1, 2, 4, 3))
    XDs = WD.reshape(NCORES, NCHUNK, P, LC)
    XAs = WS.reshape(NCORES, NCHUNK, P, LC)
    return [{"xl": XLs[k], "xd": XDs[k], "xa": XAs[k]} for k in range(NCORES)]


def _run(in_maps, trace=False, **kw):
    nc = _get_nc()
    return run_bass_kernel_spmd(nc, in_maps, list(range(NCORES)), trace=trace, **kw)


def kernel(inputs, targets, cluster_ids, sample_weight, pair_i, pair_j):
    in_maps = _prepare(inputs, targets, cluster_ids, sample_weight, pair_i, pair_j)
    res = _run(in_maps)
    tl = 0.0
    tw = 0.0
    for k in range(NCORES):
        o = res.results[k]["out"]
        tl += float(o[0, 0:LC].sum(dtype=np.float64))
        tw += float(o[0, LC:2 * LC].sum(dtype=np.float64))
    # the 0.5 pair-weight factor cancels in the ratio; fold it into eps
    return np.float32(tl / (tw + 2 * EPS))
